# revision 11
# baseline (speedup 1.0000x reference)
"""Bilinear kernel for Trainium2 (Bass/Tile), SPMD over 8 NeuronCores.

out[s, i, j] = sum_{d,e} tensor1[s,i,d] * kernel[d,e] * tensor0[s,j,e] + bias

Sharding: data-parallel over the S (=8) sample axis, one sample per core.
Per core (N=2048, D=256):
    qt0T[d, j] = sum_e kernel[d, e] * tensor0[j, e]        (= K @ t0^T)
    out[i, j]  = sum_d tensor1[i, d] * qt0T[d, j]          (= t1 @ qt0T)
bias (a scalar) is added on the host after the gather.

Inputs are cast to bf16 on the host and loaded PRE-TRANSPOSED with the
DMA-transpose XBAR: the contraction dim lands on SBUF partitions, so
the tensor engine runs zero transposes — 16 qt0 matmuls plus the
128-matmul GEMM, all bf16 (1 row/cycle) into fp32 PSUM. bf16 halves
input reads and output writes; total error ~5e-3 vs the 2e-2 gate.

XBAR rules learned on HW:
- Two XBAR transposes in flight on both HWDGE rings corrupt each other,
  and the tile scheduler serializes every XBAR against ALL other DMA
  traffic (global DMA lock), so the chain is a DMA blackout window.
- Therefore: ONE pure-XBAR chain on the sync ring, nothing DMA'd before
  it, ordered by consumer: kT -> t0 j-half 0 (qt0 chunks 0/1, jh0
  sweep) -> t1 i-half 0 -> t1 i-half 1 -> t0 j-half 1 (jh1's qt0).

Program order qt0(c0,c1) -> jh0 sweep -> qt0(c2,c3) -> jh1 sweep keeps
every engine FIFO free of waits on late inputs. Per (jh, i): one
[128,1024] fp32 PSUM tile (4 matmuls, db-outer), whole-tile cast to
bf16 by DVE/ACT alternating; adjacent i-tiles pair into one 512 KB
store. Stores rotate over three DMA paths (scalar HWDGE, sync HWDGE,
gpsimd SWDGE) so the 8.4 MB/core write stream drains without tailing
the kernel; the final tiles go out as single stores on separate rings.
"""

import os
import sys

for _p in ("/root/.axon_site/_ro/trn_rl_repo", "/opt/trn_rl_repo"):
    # later inserts win: prefer /opt/trn_rl_repo (writable, carries the
    # antenv.axon_hooks NTFF shim), fall back to the read-only axon copy
    if os.path.isdir(_p) and _p not in sys.path:
        sys.path.insert(0, _p)

import numpy as np

S, N, D = 8, 2048, 256
P = 128
NCORES = 8
NT = N // P   # 16 row tiles of tensor1/output
DB = D // P   # 2 blocks of the contraction dim
NJ = N // 512  # 4 j chunks of 512

_CACHE = {}

LAST_RESULTS = None  # test.py introspection (exec_time_ns etc.)


def _build_nc():
    import concourse.bacc as bacc
    import concourse.mybir as mybir
    import concourse.tile as tile
    from concourse.bass import ts

    f32 = mybir.dt.float32
    bf16 = mybir.dt.bfloat16

    nc = bacc.Bacc(
        "TRN2",
        target_bir_lowering=False,
        debug=False,
        num_devices=NCORES,
    )

    t0_d = nc.dram_tensor("tensor0", [N, D], bf16, kind="ExternalInput")
    t1_d = nc.dram_tensor("tensor1", [N, D], bf16, kind="ExternalInput")
    k_d = nc.dram_tensor("kernel", [D, D], bf16, kind="ExternalInput")
    out_d = nc.dram_tensor("out", [N, N], bf16, kind="ExternalOutput")

    NWARM = 3  # junk matmuls bridge the PE from preamble to first real work
    NH = N // 2

    with tile.TileContext(nc) as tc:
        with (
            tc.tile_pool(name="const", bufs=1) as const,
            tc.tile_pool(name="tposed", bufs=1) as tposed,
            tc.tile_pool(name="stage", bufs=4) as stage,
            tc.tile_pool(name="ps", bufs=4, space="PSUM") as psP,
        ):
            # ---- the pure XBAR chain (see module docstring).
            # kT[:, e, d] = K[d, e-blk].T ; t0T[:, e, j] = t0[j, e-blk].T ;
            # t1T[:, db, i] = t1[i, db-blk].T
            kT = tposed.tile([P, DB, D], bf16)
            t0T = tposed.tile([P, DB, N], bf16)
            t1T = tposed.tile([P, DB, N], bf16)
            for e in range(DB):
                nc.sync.dma_start_transpose(out=kT[:, e, :], in_=k_d[:, ts(e, P)])
            for e in range(DB):
                nc.sync.dma_start_transpose(
                    out=t0T[:, e, ts(0, NH)], in_=t0_d[ts(0, NH), ts(e, P)]
                )
            for h in range(2):
                for db in range(DB):
                    nc.sync.dma_start_transpose(
                        out=t1T[:, db, ts(h, NH)],
                        in_=t1_d[ts(h, NH), ts(db, P)],
                    )
            for e in range(DB):
                nc.sync.dma_start_transpose(
                    out=t0T[:, e, ts(1, NH)], in_=t0_d[ts(1, NH), ts(e, P)]
                )

            # ---- HAM warmup: junk matmuls with no DMA dependency.
            junk = const.tile([P, 512], f32)
            nc.vector.memset(junk[:], 1.0)
            for w in range(NWARM):
                wp = psP.tile([P, 1024], f32, tag="mm", name=f"warm{w}")
                nc.tensor.matmul(
                    wp[:, 0:512], junk[:, 0:P], junk[:], start=True, stop=True
                )

            # ---- qt0T[d, j] = sum_e K[d,e] t0[j,e], 512 j-columns at a time.
            qt0T = tposed.tile([P, DB, NJ, 512], bf16)

            def qt0_chunk(c):
                for db in range(DB):
                    ps = psP.tile([P, 1024], f32, tag="mm", name=f"q{c}_{db}")
                    for e in range(DB):
                        nc.tensor.matmul(
                            ps[:, 0:512],
                            kT[:, e, ts(db, P)],
                            t0T[:, e, ts(c, 512)],
                            start=(e == 0),
                            stop=(e == DB - 1),
                        )
                    if db % 2 == 0:
                        nc.vector.tensor_copy(qt0T[:, db, c, :], ps[:, 0:512])
                    else:
                        nc.scalar.copy(qt0T[:, db, c, :], ps[:, 0:512])

            # ---- jh-major big GEMM (see module docstring).
            def gemm_half(jh):
                for ip in range(NT // 2):
                    last = jh == 1 and ip >= NT // 2 - 2
                    ot = stage.tile(
                        [P, 2, 1024], bf16, tag="ot", name=f"ot{jh}_{ip}"
                    )
                    for t in range(2):
                        i = ip * 2 + t
                        pm = psP.tile([P, 1024], f32, tag="mm", name=f"pm{jh}_{i}")
                        for db in range(DB):
                            for j2 in range(2):
                                j = jh * 2 + j2
                                nc.tensor.matmul(
                                    pm[:, ts(j2, 512)],
                                    t1T[:, db, ts(i, P)],
                                    qt0T[:, db, j, :],
                                    start=(db == 0),
                                    stop=(db == DB - 1),
                                )
                        if t == 0:
                            nc.vector.tensor_copy(ot[:, 0, :], pm[:])
                        else:
                            nc.scalar.copy(ot[:, 1, :], pm[:])
                        if last:
                            # tail: single-tile stores fan out across rings
                            eng = (nc.scalar, nc.sync, nc.gpsimd, nc.scalar)[
                                (ip % 2) * 2 + t
                            ]
                            eng.dma_start(
                                out=out_d[ts(i, P), ts(jh, 1024)],
                                in_=ot[:, t, :],
                            )
                    if not last:
                        dst = out_d[ts(ip, 2 * P), ts(jh, 1024)].rearrange(
                            "(t p) f -> p t f", p=P
                        )
                        eng = (nc.scalar, nc.sync, nc.gpsimd)[ip % 3]
                        eng.dma_start(out=dst, in_=ot[:])

            qt0_chunk(0)
            qt0_chunk(1)
            gemm_half(0)
            qt0_chunk(2)
            qt0_chunk(3)
            gemm_half(1)

    nc.compile()
    return nc


def _get_nc():
    if "nc" not in _CACHE:
        _CACHE["nc"] = _build_nc()
    return _CACHE["nc"]


def kernel(tensor0, tensor1, kernel, bias):
    global LAST_RESULTS
    import ml_dtypes

    nc = _get_nc()
    from concourse.bass_utils import run_bass_kernel_spmd

    bf = ml_dtypes.bfloat16
    t0 = np.ascontiguousarray(np.asarray(tensor0, dtype=np.float32).astype(bf))
    t1 = np.ascontiguousarray(np.asarray(tensor1, dtype=np.float32).astype(bf))
    k = np.ascontiguousarray(np.asarray(kernel, dtype=np.float32).astype(bf))
    b = float(np.asarray(bias, dtype=np.float32).reshape(-1)[0])

    in_maps = [
        {"tensor0": t0[s], "tensor1": t1[s], "kernel": k} for s in range(NCORES)
    ]
    res = run_bass_kernel_spmd(nc, in_maps, list(range(NCORES)))
    LAST_RESULTS = res
    out = np.stack(
        [np.asarray(res.results[s]["out"]).astype(np.float32) for s in range(NCORES)],
        axis=0,
    )
    if b != 0.0:
        out = out + np.float32(b)
    return out.astype(np.float32, copy=False)
